# revision 34
# baseline (speedup 1.0000x reference)
"""Bass/Trainium2 kernel for BertSelfAttention with relation (graph) embeddings.

Reference computation (per batch b):
    q = x @ Wq.T + bq        k = x @ Wk.T + bk        v = x @ Wv.T + bv
    (split into H=16 heads of D=64)
    dp_k[0] = dp_v[0] = 0  (padding_idx)
    scores  = q.k/sqrt(D) + q.dp_k[g[q,k]] + mask
    probs   = softmax(scores)
    ctx     = probs @ v + sum_k probs * dp_v[g]

Sharding: data-parallel over batch (8 cores, one batch element each).

Design notes (final):
  - everything bf16 on the PE: fp32 matmuls run 3-4x slower (fp32_mode=HIGH)
  - W^T and X^T are prepared/swizzled host-side into the exact SBUF tile
    layout, so every input DMA is a full-bandwidth contiguous transfer
  - scores are computed TRANSPOSED (psT[k,q] = K^T-slice.T @ Q^T) so that
    exp(psT) yields E^T directly and no E transposes are ever needed; the
    relation score term rides the same PSUM accumulation as
    M_e-tile.T @ diag(r_e[q]) matmuls; the attention mask (k-indexed)
    becomes a per-partition bias on the exp activation
  - relation value term folded into the PV matmul: ctx^T += dpv_e-bcast.T @
    (M_e^T . E^T) with the masked products formed by a bitwise AND of E^T
    bits with 0xFFFF/0 uint16 bit-masks (exact, int32-paired on the DVE);
    the softmax denominator Z rides along as a ones column of V
  - head-paired software pipeline (scores -> A-products -> PV -> out), each
    stage lagged one head-pair so PE inputs are always ready; QK matmuls of
    a pair sit on row groups 0-1/2-3 and overlap on the PE array
  - normalization folded into the final PSUM eviction via activation scale
"""

import numpy as np
import ml_dtypes

import concourse.bass as bass
import concourse.mybir as mybir
import concourse.tile as tile
from concourse import bacc
from concourse.bass_utils import run_bass_kernel_spmd
from concourse.masks import make_identity

F32 = mybir.dt.float32
BF16 = mybir.dt.bfloat16
I32 = mybir.dt.int32
Alu = mybir.AluOpType
Act = mybir.ActivationFunctionType

B, S, HID, H, D = 8, 512, 1024, 16, 64
NCORES = 8
NQT = S // 128    # 4 q-tiles (also k-tiles) per sequence
NIT = HID // 128  # 8 tiles over the hidden dim


def build_module(with_mask, with_bias):
    nc = bacc.Bacc(
        "TRN2",
        target_bir_lowering=False,
        debug=False,
        enable_asserts=False,
        num_devices=NCORES,
    )
    xt_in = nc.dram_tensor("xt", [128, NIT, S], BF16, kind="ExternalInput").ap()
    I8 = mybir.dt.int8
    g_in = nc.dram_tensor("g", [S, S], I8, kind="ExternalInput").ap()
    gt_in = nc.dram_tensor("gt", [S, S], I8, kind="ExternalInput").ap()
    wqt_in = nc.dram_tensor("wqt", [NIT, 128, NIT, 128], BF16,
                            kind="ExternalInput").ap()
    wkt_in = nc.dram_tensor("wkt", [NIT, 128, NIT, 128], BF16,
                            kind="ExternalInput").ap()
    wvt_in = nc.dram_tensor("wvt", [2, 128, NIT, 512], BF16,
                            kind="ExternalInput").ap()
    dpkbd_in = nc.dram_tensor("dpkbd", [128, 4], BF16, kind="ExternalInput").ap()
    dpv_in = nc.dram_tensor("dpv", [2, D], BF16, kind="ExternalInput").ap()
    mask_in = bias_in = None
    if with_mask:
        mask_in = nc.dram_tensor("mask", [128, NQT], F32, kind="ExternalInput").ap()
    if with_bias:
        # bq/8, bk in column layout [128, NIT]; bv natural row [1, HID]
        bias_in = (
            nc.dram_tensor("bqc", [128, NIT], F32, kind="ExternalInput").ap(),
            nc.dram_tensor("bkc", [128, NIT], F32, kind="ExternalInput").ap(),
            nc.dram_tensor("bvr", [1, HID], BF16, kind="ExternalInput").ap(),
        )
    out_dram = nc.dram_tensor("out", [S, HID], BF16, kind="ExternalOutput").ap()

    with tile.TileContext(nc) as tc:
        build_kernel(nc, tc, xt_in, g_in, gt_in, wqt_in, wkt_in, wvt_in,
                     dpkbd_in, dpv_in, mask_in, bias_in, out_dram)
    nc.compile()
    return nc


def build_kernel(nc, tc, xt_in, g_in, gt_in, wqt_in, wkt_in, wvt_in,
                 dpkbd_in, dpv_in, mask_in, bias_in, out_dram):
    from contextlib import ExitStack
    ctx = ExitStack()
    PP = ctx.enter_context(tc.tile_pool(name="persist", bufs=1))
    WP = ctx.enter_context(tc.tile_pool(name="wpool", bufs=2))
    GP = ctx.enter_context(tc.tile_pool(name="gpool", bufs=2))
    TB = ctx.enter_context(tc.tile_pool(name="etpool", bufs=5))
    AB = ctx.enter_context(tc.tile_pool(name="apool", bufs=4))
    DG = ctx.enter_context(tc.tile_pool(name="dgpool", bufs=18))
    CT = ctx.enter_context(tc.tile_pool(name="ctpool", bufs=4))
    # PSUM (8 banks): scores 3 + E^T-stage 2 + ctx 1 + out-transpose 2.
    # The projection pool is scoped and released before ET/PC/PX open.
    PS = ctx.enter_context(tc.tile_pool(name="ps_s", bufs=5, space="PSUM"))

    # ---- X^T (host-swizzled to the exact SBUF layout); split per i-tile
    # so the first projection matmul can start after 64KB, not 512KB ----
    xt = PP.tile([128, NIT, S], BF16)
    for it_ in range(NIT):
        nc.sync.dma_start(out=xt[:, it_, :], in_=xt_in[:, it_, :])

    # ---- constants ----
    identb = PP.tile([128, 128], BF16)
    make_identity(nc, identb[:])
    wrm = PS.tile([128, 128], F32, tag="ps_s")
    for i in range(12):
        nc.tensor.matmul(wrm[:], identb[:], identb[:], start=(i == 0),
                         stop=(i == 11))
    allones_i32 = PP.tile([128, 1], I32)
    nc.vector.memset(allones_i32[:], -1)
    dpkbd = PP.tile([128, 4], BF16)
    nc.sync.dma_start(out=dpkbd[:], in_=dpkbd_in)
    # dpv rows broadcast to all 128 partitions: [128, 2, D]
    dpvbc = PP.tile([128, 2, D], BF16)
    dpv_b = bass.AP(tensor=dpv_in.tensor, offset=0, ap=[[0, 128], [D, 2], [1, D]])
    nc.sync.dma_start(out=dpvbc[:], in_=dpv_b)
    if mask_in is not None:
        maskc = PP.tile([128, NQT], F32)
        nc.sync.dma_start(out=maskc[:], in_=mask_in)
    if bias_in is not None:
        bqc = PP.tile([128, NIT], F32)
        nc.sync.dma_start(out=bqc[:], in_=bias_in[0])
        bkc = PP.tile([128, NIT], F32)
        nc.sync.dma_start(out=bkc[:], in_=bias_in[1])
        bvr = PP.tile([1, HID], BF16)
        nc.sync.dma_start(out=bvr[:], in_=bias_in[2])
        ones_rowv = PP.tile([1, 128], BF16)
        nc.vector.memset(ones_rowv[:], 1.0)

    # ---- masks: natural in bf16 (matmul rhs), transposed as uint16
    # bit-masks 0xFFFF/0x0000 (for bitwise-AND masking of E^T) ----
    U16 = mybir.dt.uint16
    m1 = PP.tile([128, NQT, S], BF16)
    m2 = PP.tile([128, NQT, S], BF16)
    m1t = PP.tile([128, NQT, S], U16)
    m2t = PP.tile([128, NQT, S], U16)
    for qt in range(NQT):
        gt_ = GP.tile([128, S], mybir.dt.int8, tag="g")
        nc.sync.dma_start(out=gt_[:], in_=g_in[128 * qt:128 * (qt + 1), :])
        nc.vector.tensor_scalar(out=m1[:, qt, :], in0=gt_[:], scalar1=1,
                                scalar2=None, op0=Alu.is_equal)
        nc.vector.tensor_scalar(out=m2[:, qt, :], in0=gt_[:], scalar1=2,
                                scalar2=None, op0=Alu.is_equal)
    for kt in range(NQT):
        gt_ = GP.tile([128, S], mybir.dt.int8, tag="g")
        nc.gpsimd.dma_start(out=gt_[:], in_=gt_in[128 * kt:128 * (kt + 1), :])
        nc.vector.tensor_scalar(out=m1t[:, kt, :], in0=gt_[:], scalar1=1,
                                scalar2=65535, op0=Alu.is_equal, op1=Alu.mult)
        nc.vector.tensor_scalar(out=m2t[:, kt, :], in0=gt_[:], scalar1=2,
                                scalar2=65535, op0=Alu.is_equal, op1=Alu.mult)

    # ---- projections (scoped PSUM pool, released before head phases) ----
    qt_sb = PP.tile([128, NIT, S], BF16)   # Q^T/8: [feature, seq]
    kt_sb = PP.tile([128, NIT, S], BF16)   # K^T
    vb = PP.tile([128, NQT, H, D + 1], BF16)  # V natural + ones column

    PBIG = tc.alloc_tile_pool(name="ps_proj", bufs=2, space="PSUM")
    for wi, (w_in, o_sb, scale) in enumerate(((wqt_in, qt_sb, 0.125),
                                              (wkt_in, kt_sb, 1.0))):
        for t in range(NIT):
            wt = WP.tile([128, NIT, 128], BF16, tag="wqk")
            nc.scalar.dma_start(out=wt[:, 0:4, :], in_=w_in[t, :, 0:4, :])
            nc.scalar.dma_start(out=wt[:, 4:8, :], in_=w_in[t, :, 4:8, :])
            ps = PBIG.tile([128, S], F32, tag="psbig")
            for it in range(NIT):
                nc.tensor.matmul(ps[:], wt[:, it, :], xt[:, it, :],
                                 start=(it == 0), stop=(it == NIT - 1))
            if bias_in is not None:
                bcol = (bqc if wi == 0 else bkc)[:, t:t + 1]
                nc.scalar.activation(o_sb[:, t, :], ps[:], Act.Identity,
                                     bias=bcol, scale=scale)
            else:
                nc.scalar.activation(o_sb[:, t, :], ps[:], Act.Identity,
                                     scale=scale)

    # rcols: r_e[q] for all (t, qt) pairs in one PSUM bank
    # layout [128, NIT, NQT, 4]; cols (2*(h%2)+e-1)
    psr = PS.tile([128, NIT, NQT, 4], F32, tag="ps_s")
    for t in range(NIT):
        for qt in range(NQT):
            nc.tensor.matmul(psr[:, t, qt, :],
                             qt_sb[:, t, 128 * qt:128 * (qt + 1)], dpkbd[:],
                             start=(t == 0 and qt == 0),
                             stop=(t == NIT - 1 and qt == NQT - 1))
    rcol = PP.tile([128, NIT, NQT, 4], F32)
    nc.vector.tensor_copy(rcol[:], psr[:])

    # V in natural layout [s, o] via lhsT = x^T
    for oc in range(2):
        wt = WP.tile([128, NIT, 512], BF16, tag="wv")
        nc.scalar.dma_start(out=wt[:], in_=wvt_in[oc])
        for st in range(NQT):
            ps = PBIG.tile([128, 512], F32, tag="psbig")
            for it in range(NIT):
                nc.tensor.matmul(ps[:], xt[:, it, 128 * st:128 * (st + 1)],
                                 wt[:, it, :],
                                 start=(it == 0),
                                 stop=(it == NIT - 1 and bias_in is None))
            if bias_in is not None:
                nc.tensor.matmul(ps[:], ones_rowv[:],
                                 bvr[:, 512 * oc:512 * (oc + 1)],
                                 start=False, stop=True)
            nc.scalar.copy(
                vb[:, st, 8 * oc:8 * (oc + 1), 0:D],
                ps[:].rearrange("p (h d) -> p h d", d=D))
    # ones column for the Z row of ctx^T
    nc.gpsimd.memset(vb[:, :, :, D:D + 1], 1.0)
    PBIG.release()
    PC = ctx.enter_context(tc.tile_pool(name="ps_c", bufs=1, space="PSUM"))
    PX = ctx.enter_context(tc.tile_pool(name="ps_x", bufs=2, space="PSUM"))

    # ---- attention: software-pipelined over heads ----
    # Stages (lagged so every PE instruction's inputs are >= 1 head old):
    #   S(h): transposed scores psT[k,q] + exp -> E^T directly (no E
    #         transposes at all): QK part is K^T-slice.T @ Q^T-full; the
    #         relation term is M_e-tile.T @ diag(r_e) per (kt, qt); the
    #         attention mask (k-indexed) rides the exp bias per-partition
    #   A(h): A_e^T mask products      P(h): PV/relval MMs + evict
    #   O(h): out-transposes + normalize
    osb = PP.tile([128, NQT, HID], BF16)
    state = {}

    def phase_scores_pair(p):
        # heads 2p, 2p+1 share t=p; their QK matmuls use row groups 0-1 /
        # 2-3 and run concurrently when adjacent in the PE stream
        t = p
        etbs = []
        dgs = {}
        for ho in range(2):
            e0 = 2 * ho
            for qt in range(NQT):
                for e in range(2):
                    dg = DG.tile([128, 128], BF16, tag="dg")
                    nc.vector.tensor_scalar(
                        out=dg[:], in0=identb[:],
                        scalar1=rcol[:, t, qt, e0 + e:e0 + e + 1],
                        scalar2=None, op0=Alu.mult)
                    dgs[(ho, qt, e)] = dg
            etbs.append(TB.tile([128, NQT, S], BF16, tag="etb",
                                    name=f"etb{ho}"))
        for kt in range(NQT):
            pss = []
            for ho in range(2):
                po = D * ho
                ps = PS.tile([128, S], F32, tag="ps_s", name=f"psT{ho}")
                nc.tensor.matmul(ps[:],
                                 kt_sb[po:po + D, t, 128 * kt:128 * (kt + 1)],
                                 qt_sb[po:po + D, t, :], start=True, stop=False)
                pss.append(ps)
            for ho in range(2):
                ps = pss[ho]
                for qt in range(NQT):
                    for e in range(2):
                        nc.tensor.matmul(
                            ps[:, 128 * qt:128 * (qt + 1)],
                            (m1 if e == 0 else m2)[:, qt,
                                                   128 * kt:128 * (kt + 1)],
                            dgs[(ho, qt, e)][:],
                            start=False, stop=(qt == NQT - 1 and e == 1))
                if mask_in is not None:
                    nc.scalar.activation(etbs[ho][:, kt, :], ps[:], Act.Exp,
                                         bias=maskc[:, kt:kt + 1])
                else:
                    nc.scalar.activation(etbs[ho][:, kt, :], ps[:], Act.Exp)
        state[2 * p] = {"etb": etbs[0]}
        state[2 * p + 1] = {"etb": etbs[1]}

    def phase_aprod(h):
        st = state[h]
        etb = st["etb"]
        # A_e^T = M_e^T . E^T as bitwise AND of E^T bits with the 0xFFFF
        # bit-masks, processed as int32 pairs (half the element count)
        a1t = AB.tile([128, NQT, S], BF16, tag="a1")
        a2t = AB.tile([128, NQT, S], BF16, tag="a2")
        for at, mt in ((a1t, m1t), (a2t, m2t)):
            nc.vector.scalar_tensor_tensor(
                out=at[:].bitcast(I32), in0=mt[:].bitcast(I32),
                scalar=allones_i32[:, 0:1], in1=etb[:].bitcast(I32),
                op0=Alu.bitwise_and, op1=Alu.bitwise_and)
        st.update(a1t=a1t, a2t=a2t)

    def phase_pv(h):
        st = state[h]
        etb, a1t, a2t = st["etb"], st["a1t"], st["a2t"]
        # ctx^T (+Z row) = [V|1]^T.T @ E^T + sum_e dpv_e-bcast.T @ A_e^T
        psC = PC.tile([D + 1, S], F32, tag="psc")
        for kt in range(NQT):
            nc.tensor.matmul(psC[:], vb[:, kt, h, :], etb[:, kt, :],
                             start=(kt == 0), stop=False)
        for e in range(2):
            at = a1t if e == 0 else a2t
            for kt in range(NQT):
                nc.tensor.matmul(psC[0:D, :], dpvbc[:, e, :], at[:, kt, :],
                                 start=False,
                                 stop=(e == 1 and kt == NQT - 1))
        cts = CT.tile([D + 1, S], BF16, tag="cts")
        nc.vector.tensor_copy(cts[:], psC[:])
        st["cts"] = cts

    def phase_out(h):
        cts = state.pop(h)["cts"]
        # 4 transposes share one PSUM tile (68-elem stride keeps 8B
        # alignment); one strided reciprocal covers all 4 Z columns
        psX4 = PX.tile([128, NQT, 68], BF16, tag="psx")
        for qt in range(NQT):
            nc.tensor.transpose(psX4[:, qt, 0:D + 1],
                                cts[:, 128 * qt:128 * (qt + 1)],
                                identb[0:D + 1, 0:D + 1])
        rz4 = DG.tile([128, NQT, 1], F32, tag="rz")
        nc.vector.reciprocal(rz4[:], psX4[:, :, D:D + 1])
        for qt in range(NQT):
            nc.scalar.activation(osb[:, qt, D * h:D * (h + 1)],
                                 psX4[:, qt, 0:D],
                                 Act.Identity, scale=rz4[:, qt, 0:1])

    NP = H // 2
    for i in range(NP + 3):
        if i < NP:
            phase_scores_pair(i)
        if 0 <= i - 1 < NP:
            for ho in range(2):
                phase_aprod(2 * (i - 1) + ho)
        if 0 <= i - 2 < NP:
            for ho in range(2):
                phase_pv(2 * (i - 2) + ho)
        if 0 <= i - 3 < NP:
            for ho in range(2):
                phase_out(2 * (i - 3) + ho)

    for qt in range(NQT):
        nc.sync.dma_start(out=out_dram[128 * qt:128 * (qt + 1), :],
                          in_=osb[:, qt, :])
    ctx.close()


_NC = None
_NC_KEY = None


def _get_module(with_mask=False, with_bias=False):
    global _NC, _NC_KEY
    key = (with_mask, with_bias)
    if _NC is None or _NC_KEY != key:
        _NC = build_module(with_mask, with_bias)
        _NC_KEY = key
    return _NC


def make_in_maps(hidden_states, attention_mask, graph_emb, Wq, bq, Wk, bk,
                 Wv, bv, dp_k, dp_v):
    with_mask = bool(np.any(np.asarray(attention_mask)))
    with_bias = bool(np.any(bq) or np.any(bk) or np.any(bv))

    bf = ml_dtypes.bfloat16
    x = np.ascontiguousarray(np.asarray(hidden_states), dtype=np.float32)
    g = np.ascontiguousarray(np.asarray(graph_emb), dtype=np.int8)

    # 8 * dp_k[1:3]^T replicated in both 64-row halves as block-diagonal
    # [128, 4]: rows 0:64 cols 0:2 = head-even, rows 64:128 cols 2:4 = head-odd
    dpk = np.asarray(dp_k, dtype=np.float32)
    dpkbd = np.zeros((128, 4), dtype=np.float32)
    dpkbd[0:D, 0:2] = 8.0 * dpk[1:3].T
    dpkbd[D:128, 2:4] = 8.0 * dpk[1:3].T

    def sw_qk(W):
        # [o, i] -> W.T tiles in exact SBUF order: [t_o, p_i, it, o128]
        A = np.asarray(W, dtype=np.float32).T.reshape(NIT, 128, NIT, 128)
        return np.ascontiguousarray(A.transpose(2, 1, 0, 3)).astype(bf)

    def sw_v(W):
        A = np.asarray(W, dtype=np.float32).T.reshape(NIT, 128, 2, 512)
        return np.ascontiguousarray(A.transpose(2, 1, 0, 3)).astype(bf)

    shared = {
        "wqt": sw_qk(Wq),
        "wkt": sw_qk(Wk),
        "wvt": sw_v(Wv),
        "dpkbd": dpkbd.astype(bf),
        "dpv": np.asarray(dp_v, dtype=np.float32)[1:3].astype(bf),
    }
    if with_mask:
        shared_mask = np.asarray(attention_mask, dtype=np.float32)
    if with_bias:
        shared["bqc"] = np.ascontiguousarray(
            (np.asarray(bq, dtype=np.float32) / 8.0).reshape(NIT, 128).T)
        shared["bkc"] = np.ascontiguousarray(
            np.asarray(bk, dtype=np.float32).reshape(NIT, 128).T)
        shared["bvr"] = np.asarray(bv, dtype=np.float32).reshape(1, HID).astype(bf)

    in_maps = []
    for c in range(NCORES):
        m = {
            "xt": np.ascontiguousarray(
                x[c].T.reshape(NIT, 128, S).transpose(1, 0, 2)).astype(bf),
            "g": g[c],
            "gt": np.ascontiguousarray(g[c].T),
            **shared,
        }
        if with_mask:
            m["mask"] = np.ascontiguousarray(
                shared_mask[c].reshape(NQT, 128).T).astype(np.float32)
        in_maps.append(m)
    return in_maps, with_mask, with_bias


def kernel(**inputs):
    in_maps, with_mask, with_bias = make_in_maps(**inputs)
    nc = _get_module(with_mask, with_bias)
    res = run_bass_kernel_spmd(nc, in_maps, list(range(NCORES)))
    out = np.stack([res.results[c]["out"] for c in range(NCORES)], axis=0)
    return out.astype(np.float32)


if __name__ == "__main__":
    rng = np.random.default_rng(0)
    inputs = {
        "hidden_states": rng.standard_normal((B, S, HID)).astype(np.float32),
        "attention_mask": np.zeros((B, 1, 1, S), np.float32),
        "graph_emb": rng.integers(0, 3, (B, S, S)).astype(np.int32),
        "Wq": (rng.standard_normal((HID, HID)) * 0.02).astype(np.float32),
        "bq": np.zeros(HID, np.float32),
        "Wk": (rng.standard_normal((HID, HID)) * 0.02).astype(np.float32),
        "bk": np.zeros(HID, np.float32),
        "Wv": (rng.standard_normal((HID, HID)) * 0.02).astype(np.float32),
        "bv": np.zeros(HID, np.float32),
        "dp_k": (rng.standard_normal((3, D)) * 0.02).astype(np.float32),
        "dp_v": (rng.standard_normal((3, D)) * 0.02).astype(np.float32),
    }
    out = kernel(**inputs)
    print("out", out.shape, out.dtype, float(np.abs(out).max()))
